# revision 1
# baseline (speedup 1.0000x reference)
"""Trainium2 Bass kernel: 3-layer GNN message passing (atom embedding).

Data-parallel over the B*N=400000 point axis across 8 NeuronCores.
Layout on chip: channels-on-partitions. Per point-tile of T=512 points,
a feature tile F[119, T] holds:
  rows 0-95   atomtypes  (k*6+c, neighbor-major)
  rows 96-111 dist       (k)
  rows 112-117 point embedding (updated per layer)
  row  118    constant 1.0 (folds the b1 bias into matmul1)
Matmul1 uses a block-diagonal packed lhsT [119, 104] per neighbor-half
(8 neighbors x 13 output channels); the emb/bias rows contribute to every
neighbor block, so no broadcast op is ever needed. Both halves land in
one two-bank PSUM tile and a single fused ScalarE Prelu [104, 1024]
applies bias+LeakyReLU while evacuating. Matmul2 stacks W2 per neighbor
(104->6) and folds the neighbor-sum into the PSUM contraction +
accumulation. GroupNorm is batched across GB=14 point-tiles into
[84, T] tensors (stats via a block-diagonal averaging matmul, rsqrt via
one Abs_reciprocal_sqrt, scale/shift+LeakyReLU fused into one Prelu).
Compute-engine APs must start at 32-aligned partitions on TRN2; all
unaligned row packing (stage assembly, per-layer emb refresh from the
batched E tensor) uses SBUF->SBUF DMA, which has no such constraint.
"""
import sys

sys.path.insert(0, "/opt/trn_rl_repo")

import numpy as np

D = 6
K = 16
N_LAYERS = 3
C_IN = 13
EPS = 1e-5
SLOPE = 0.2

N_CORES = 8
T = 512            # points per tile (PSUM bank = 512 fp32)
PC = 50000         # points per core
NT = 98            # tiles per core
PP = NT * T        # padded points per core = 50176
GB = 14            # point-tiles per groupnorm batch (84 partitions)
SROWS = 6 * GB     # 84

F_ROWS = 119       # 96 atom + 16 dist + 6 emb + 1 ones


def _pack_weights(W1, b1, W2, b2, gw, gb):
    """Build the packed lhsT / const tensors (host side, a few KB)."""
    lhsT1 = np.zeros((N_LAYERS, 2, F_ROWS, 104), np.float32)
    for i in range(N_LAYERS):
        for half in range(2):
            L = lhsT1[i, half]
            for k8 in range(8):
                k = half * 8 + k8
                cols = slice(k8 * 13, k8 * 13 + 13)
                L[k * 6:(k + 1) * 6, cols] = W1[i, 6:12, :]   # atom rows
                L[96:102, cols] = W1[i, 0:6, :]               # emb rows
                L[102 + k, cols] = W1[i, 12, :]               # dist row
                L[118, cols] = b1[i]                          # bias row
    lhsT1_flat = np.concatenate(
        [lhsT1[i, h] for i in range(N_LAYERS) for h in range(2)], axis=1
    )  # [119, 624]

    lhsT2 = np.zeros((104, N_LAYERS * 6), np.float32)
    for i in range(N_LAYERS):
        for k8 in range(8):
            lhsT2[k8 * 13:k8 * 13 + 13, i * 6:(i + 1) * 6] = W2[i]

    G1 = np.kron(np.eye(2 * GB, dtype=np.float32),
                 np.ones((3, 3), np.float32) / 3.0)  # [6*GB, 6*GB]

    cst = np.zeros((6, 9), np.float32)
    for i in range(N_LAYERS):
        cst[:, i] = 16.0 * b2[i]
        cst[:, 3 + i] = gw[i]
        cst[:, 6 + i] = gb[i]
    cst = np.tile(cst, (GB, 1))  # [6*GB, 9]
    return lhsT1_flat, lhsT2, G1, cst


def _build_nc():
    import concourse.bass as bass
    import concourse.bacc as bacc
    import concourse.mybir as mybir
    from concourse import tile

    bass_ds = bass.ds
    F32 = mybir.dt.float32
    AF = mybir.ActivationFunctionType
    OP = mybir.AluOpType

    import time as _time
    print(f"[kernel] build start {_time.time():.1f}", flush=True)
    nc = bacc.Bacc("TRN2", target_bir_lowering=False)
    atom_e = nc.declare_dram_parameter("atom", [96, PP], F32, isOutput=False)
    dist_e = nc.declare_dram_parameter("dist", [16, PP], F32, isOutput=False)
    l1_e = nc.declare_dram_parameter("lhsT1", [F_ROWS, 624], F32, isOutput=False)
    l2_e = nc.declare_dram_parameter("lhsT2", [104, 18], F32, isOutput=False)
    g1_e = nc.declare_dram_parameter("g1", [SROWS, SROWS], F32, isOutput=False)
    cst_e = nc.declare_dram_parameter("cst", [SROWS, 9], F32, isOutput=False)
    out_e = nc.declare_dram_parameter("out", [6, PP], F32, isOutput=True)

    with tile.TileContext(nc) as tc:
        with tc.tile_pool(name="w", bufs=1) as wp, \
             tc.tile_pool(name="f", bufs=2) as fp, \
             tc.tile_pool(name="h", bufs=4) as hp, \
             tc.tile_pool(name="g", bufs=2) as gp, \
             tc.tile_pool(name="z", bufs=2, space="PSUM") as zp, \
             tc.tile_pool(name="m", bufs=2, space="PSUM") as mp, \
             tc.tile_pool(name="s", bufs=2, space="PSUM") as sp:
            l1 = wp.tile([F_ROWS, 624], F32)
            l2 = wp.tile([104, 18], F32)
            g1 = wp.tile([SROWS, SROWS], F32)
            cst = wp.tile([SROWS, 9], F32)
            eps = wp.tile([128, 1], F32)
            nc.sync.dma_start(out=l1[:], in_=l1_e[:])
            nc.sync.dma_start(out=l2[:], in_=l2_e[:])
            nc.sync.dma_start(out=g1[:], in_=g1_e[:])
            nc.sync.dma_start(out=cst[:], in_=cst_e[:])
            nc.gpsimd.memset(eps[:], EPS)

            with tc.For_i(0, PP, GB * T) as s:
                Fs = []
                for j in range(GB):
                    F = fp.tile([F_ROWS, T], F32, tag=f"F{j}")
                    nc.sync.dma_start(out=F[0:96, :],
                                      in_=atom_e[:, bass_ds(s + j * T, T)])
                    # memset must start 32-aligned: fill [96:119) with 1.0
                    # (emb init + ones row), then dist DMA overwrites 102-117.
                    nc.vector.memset(F[96:F_ROWS, :], 1.0)
                    nc.sync.dma_start(out=F[102:118, :],
                                      in_=dist_e[:, bass_ds(s + j * T, T)])
                    Fs.append(F)
                # batched emb state [84, T]: rows 6j = tile j's embedding
                E = gp.tile([SROWS, T], F32, tag="E")
                nc.vector.memset(E[:], 1.0)

                for i in range(N_LAYERS):
                    stage = gp.tile([SROWS, T], F32, tag="stage")
                    l2s = l2[:, i * 6:(i + 1) * 6]
                    off = (i * 2) * 104
                    for j in range(GB):
                        if i > 0:
                            # refresh this tile's emb rows from E (DMA: no
                            # partition-alignment constraint)
                            nc.sync.dma_start(out=Fs[j][96:102, :],
                                              in_=E[6 * j:6 * j + 6, :])
                        # Z spans two PSUM banks: one fused Prelu evacuates
                        # both matmul1 halves (amortizes the ~352-cycle
                        # ScalarE per-op overhead over 1024 columns).
                        Z = zp.tile([128, 2 * T], F32, tag="Z")
                        nc.tensor.matmul(Z[0:104, 0:T], l1[:, off:off + 104],
                                         Fs[j][:], start=True, stop=True)
                        nc.tensor.matmul(Z[0:104, T:2 * T],
                                         l1[:, off + 104:off + 208],
                                         Fs[j][:], start=True, stop=True)
                        H = hp.tile([104, 2 * T], F32, tag="H")
                        nc.scalar.activation(H[:], Z[0:104, :], AF.Prelu,
                                             bias=0.0, scale=1.0, alpha=SLOPE)
                        msg = mp.tile([6, T], F32, tag="msg")
                        nc.tensor.matmul(msg[0:6, :], l2s, H[:, 0:T],
                                         start=True, stop=False)
                        nc.tensor.matmul(msg[0:6, :], l2s, H[:, T:2 * T],
                                         start=False, stop=True)
                        # evac + 16*b2 bias at base 0, then DMA into stage row
                        et = hp.tile([6, T], F32, tag="etmp")
                        nc.vector.tensor_scalar(et[:], msg[0:6, :],
                                                cst[0:6, i:i + 1], None, OP.add)
                        nc.sync.dma_start(out=stage[6 * j:6 * j + 6, :],
                                          in_=et[:])

                    # ---- batched GroupNorm over [84, T] ----
                    sq = gp.tile([SROWS, T], F32, tag="sq")
                    nc.scalar.activation(sq[:], stage[:], AF.Square)
                    mu = sp.tile([SROWS, T], F32, tag="mu")
                    m2 = sp.tile([SROWS, T], F32, tag="mu")
                    nc.tensor.matmul(mu[0:SROWS, :], g1[:], stage[:],
                                     start=True, stop=True)
                    nc.tensor.matmul(m2[0:SROWS, :], g1[:], sq[:],
                                     start=True, stop=True)
                    mus2 = gp.tile([SROWS, T], F32, tag="nmu2")
                    nc.scalar.activation(mus2[:], mu[0:SROWS, :], AF.Square)
                    vpe = gp.tile([SROWS, T], F32, tag="vpe")
                    nc.vector.scalar_tensor_tensor(
                        vpe[:], mus2[:], -1.0, m2[0:SROWS, :],
                        OP.mult, OP.add)
                    # vpe+eps > 0, so 1/sqrt(|x|) == rsqrt(x): one fused op
                    # shortens the serial groupnorm chain by a hop.
                    rstd = gp.tile([SROWS, T], F32, tag="rstd")
                    nc.scalar.activation(rstd[:], vpe[:],
                                         AF.Abs_reciprocal_sqrt,
                                         bias=eps[0:SROWS, 0:1], scale=1.0)
                    y = gp.tile([SROWS, T], F32, tag="y")
                    nc.vector.scalar_tensor_tensor(
                        y[:], mu[0:SROWS, :], -1.0, stage[:], OP.mult, OP.add)
                    yr = gp.tile([SROWS, T], F32, tag="yr")
                    nc.vector.tensor_mul(yr[:], y[:], rstd[:])
                    upd = gp.tile([SROWS, T], F32, tag="upd")
                    nc.scalar.activation(upd[:], yr[:], AF.Prelu,
                                         bias=cst[:, 6 + i:7 + i],
                                         scale=cst[:, 3 + i:4 + i], alpha=SLOPE)
                    nc.vector.tensor_add(E[:], E[:], upd[:])

                for j in range(GB):
                    nc.sync.dma_start(out=out_e[:, bass_ds(s + j * T, T)],
                                      in_=E[6 * j:6 * j + 6, :])

    print(f"[kernel] trace done {_time.time():.1f}", flush=True)
    nc.compile()
    print(f"[kernel] bacc compile done {_time.time():.1f}", flush=True)
    return nc


_RUNNER_CACHE = None


def get_runner():
    """Build (once) the jitted 8-core executable. Returns a dict with:
    fn(concat_inputs...) -> out jax arrays, in_names order, zero_outs."""
    global _RUNNER_CACHE
    if _RUNNER_CACHE is not None:
        return _RUNNER_CACHE
    import jax
    import numpy as _np
    from jax.sharding import Mesh, PartitionSpec
    from jax.experimental.shard_map import shard_map
    import concourse.mybir as mybir
    from concourse.bass2jax import (
        install_neuronx_cc_hook, _bass_exec_p, partition_id_tensor)

    nc = _build_nc()
    install_neuronx_cc_hook()
    partition_name = nc.partition_id_tensor.name if nc.partition_id_tensor else None
    in_names, out_names, out_avals, zero_outs = [], [], [], []
    for alloc in nc.m.functions[0].allocations:
        if not isinstance(alloc, mybir.MemoryLocationSet):
            continue
        name = alloc.memorylocations[0].name
        if alloc.kind == "ExternalInput":
            if name != partition_name:
                in_names.append(name)
        elif alloc.kind == "ExternalOutput":
            out_names.append(name)
            shape = tuple(alloc.tensor_shape)
            dtype = mybir.dt.np(alloc.dtype)
            out_avals.append(jax.core.ShapedArray(shape, dtype))
            zero_outs.append(_np.zeros(shape, dtype))
    n_params = len(in_names)
    all_in_names = in_names + out_names
    if partition_name is not None:
        all_in_names.append(partition_name)

    def _body(*args):
        operands = list(args)
        if partition_name is not None:
            operands.append(partition_id_tensor())
        return tuple(_bass_exec_p.bind(
            *operands,
            out_avals=tuple(out_avals),
            in_names=tuple(all_in_names),
            out_names=tuple(out_names),
            lowering_input_output_aliases=(),
            sim_require_finite=True,
            sim_require_nnan=True,
            nc=nc,
        ))

    devices = jax.devices()[:N_CORES]
    mesh = Mesh(_np.asarray(devices), ("core",))
    nin = n_params + len(out_names)
    sharded = jax.jit(shard_map(
        _body, mesh=mesh,
        in_specs=(PartitionSpec("core"),) * nin,
        out_specs=(PartitionSpec("core"),) * len(out_names),
        check_rep=False))
    _RUNNER_CACHE = {
        "fn": sharded, "in_names": in_names, "out_names": out_names,
        "zero_outs": zero_outs, "mesh": mesh,
    }
    return _RUNNER_CACHE


def run_cores(in_maps):
    """Run the 8-core kernel on a list of per-core input dicts."""
    import numpy as _np
    r = get_runner()
    concat_in = [
        _np.concatenate([in_maps[c][name] for c in range(N_CORES)], axis=0)
        for name in r["in_names"]
    ] + [_np.concatenate([z] * N_CORES, axis=0) for z in r["zero_outs"]]
    outs = r["fn"](*concat_in)
    res = []
    for c in range(N_CORES):
        d = {}
        for i, name in enumerate(r["out_names"]):
            full = _np.asarray(outs[i])
            rows = full.shape[0] // N_CORES
            d[name] = full[c * rows:(c + 1) * rows]
        res.append(d)
    return res


def make_in_maps(dist, atomtypes, W1, b1, W2, b2, gw, gb):
    dist = np.asarray(dist, np.float32)
    atomtypes = np.asarray(atomtypes, np.float32)
    B, N, _, _ = atomtypes.shape
    P = B * N
    assert P == N_CORES * PC

    lhsT1, lhsT2, G1, cst = _pack_weights(
        np.asarray(W1, np.float32), np.asarray(b1, np.float32),
        np.asarray(W2, np.float32), np.asarray(b2, np.float32),
        np.asarray(gw, np.float32), np.asarray(gb, np.float32))

    atom_flat = atomtypes.reshape(P, K * D)      # [400000, 96]
    dist_flat = dist.reshape(P, K)               # [400000, 16]

    in_maps = []
    for c in range(N_CORES):
        sl = slice(c * PC, (c + 1) * PC)
        a = np.zeros((PP, 96), np.float32)
        a[:PC] = atom_flat[sl]
        d = np.zeros((PP, 16), np.float32)
        d[:PC] = dist_flat[sl]
        in_maps.append({
            "atom": np.ascontiguousarray(a.T),
            "dist": np.ascontiguousarray(d.T),
            "lhsT1": lhsT1, "lhsT2": lhsT2, "g1": G1, "cst": cst,
        })
    return in_maps, (B, N)


def kernel(dist, atomtypes, W1, b1, W2, b2, gw, gb):
    in_maps, (B, N) = make_in_maps(dist, atomtypes, W1, b1, W2, b2, gw, gb)
    res = run_cores(in_maps)
    outs = [res[c]["out"][:, :PC].T for c in range(N_CORES)]
    return np.concatenate(outs, axis=0).reshape(B, N, D).astype(np.float32)


if __name__ == "__main__":
    rng = np.random.default_rng(0)
    inputs = {
        "dist": rng.random((4, 100000, 16, 1), dtype=np.float32),
        "atomtypes": rng.random((4, 100000, 16, 6), dtype=np.float32),
        "W1": rng.random((3, 13, 13), dtype=np.float32) - 0.5,
        "b1": rng.random((3, 13), dtype=np.float32) - 0.5,
        "W2": rng.random((3, 13, 6), dtype=np.float32) - 0.5,
        "b2": rng.random((3, 6), dtype=np.float32) - 0.5,
        "gw": np.ones((3, 6), np.float32),
        "gb": np.zeros((3, 6), np.float32),
    }
    out = kernel(**inputs)
    print(out.shape, out.dtype)



# revision 26
# speedup vs baseline: 1.1016x; 1.1016x over previous
"""Trainium2 Bass kernel: 3-layer GNN message passing (atom embedding).

Data-parallel over the B*N=400000 point axis across 8 NeuronCores.

Layout (channels-on-partitions, fp32 data, float32r matmuls):
  F [128, GB*T] fp32 per batch of GB=14 point-tiles; each 64-partition
  half h holds neighbors 8h..8h+7 of every point:
    rows 64h+0..5   emb (refreshed per layer from E)
    row  64h+6      const 1.0 (folds b1 into matmul1)
    rows 64h+8+6k+c atom   (k=0..7 within half)
    rows 64h+56+k   dist
  matmul1 per tile: two K=64 matmuls (one per half, identical lhsT1
  content replicated at partitions 0-63 / 64-127) -> Z [104, 2T] PSUM.
  LeakyReLU evacuation Z->H split between ScalarE (Prelu, cols 0:C1)
  and VectorE (copy to SBUF then (x*0.2) max x; DVE may read PSUM only
  once per op).
  matmul2: per-tile lhsT2 [104,84] (W2 at output cols 6j) accumulates
  all GB tiles' messages directly into one batched PSUM stage [84, T];
  the +16*b2 bias is folded into the two PSUM->SBUF reads.
  GroupNorm batched over [84, T].

  Precision: GroupNorm's var=m2-mu^2 sits at the eps=1e-5 threshold on
  this data, amplifying msg noise ~300x — ANY 16-bit quantization of
  inputs/weights/h fails the 2e-2 tolerance (bf16 -> rel 0.3, fp16 ->
  0.23, measured).  So all tensors stay fp32; the 1176 main matmuls run
  as float32r (bitcast views: full-rate rows vs 4 cycles/row for plain
  fp32, and self-loading — no per-matmul LDWEIGHTS instruction).  The
  42 GN stats matmuls stay plain fp32: float32r rounding in m2 would
  swamp var at the eps scale.

  Batches are emitted software-pipelined in groups of 2-3 (units
  (b0,i),(b1,i),(b0,i+1)...) so the per-layer serial GN chain of one
  batch overlaps the tile crunch of another instead of stalling PE.
  ~7 DMAs per batch (batched loads, one rearranged-AP scatter per half
  for the emb refresh, gpsimd out-store off the SP HWDGE FIFO).
"""
import sys

sys.path.insert(0, "/opt/trn_rl_repo")

import numpy as np

D = 6
K = 16
N_LAYERS = 3
C_IN = 13
EPS = 1e-5
SLOPE = 0.2

N_CORES = 8
T = 512            # points per tile (PSUM bank = 512 fp32)
PC = 50000         # points per core
NT = 98            # tiles per core
PP = NT * T        # padded points per core = 50176
GB = 14            # point-tiles per groupnorm batch (84 partitions)
SROWS = 6 * GB     # 84
C1 = 736           # ScalarE/VectorE split of the Z->H evacuation
NB = NT // GB      # 7 batches
# batch groups for software pipelining (2-3 batches in flight)
GROUPS = [(0, 1), (2, 3), (4, 5, 6)]
USE_F32R = False


def _pack_weights(W1, b1, W2, b2, gw, gb):
    """Build packed lhsT / const tensors (host side, a few KB)."""
    # lhsT1 [128, 3*104]: rows 0-63 = half template, 64-127 copy.
    half = np.zeros((64, N_LAYERS * 104), np.float32)
    for i in range(N_LAYERS):
        for k8 in range(8):
            cols = slice(i * 104 + k8 * 13, i * 104 + k8 * 13 + 13)
            half[0:6, cols] = W1[i, 0:6, :]          # emb rows
            half[6, cols] = b1[i]                    # ones row -> bias
            half[8 + k8 * 6:8 + k8 * 6 + 6, cols] = W1[i, 6:12, :]
            half[56 + k8, cols] = W1[i, 12, :]       # dist row
    lhsT1 = np.concatenate([half, half], axis=0)     # [128, 312]

    # lhsT2 [104, 3*GB*84]: variant (i, j) puts W2[i] at output cols
    # 6j..6j+6 for every neighbor block.
    lhsT2 = np.zeros((104, N_LAYERS * GB * 84), np.float32)
    for i in range(N_LAYERS):
        for j in range(GB):
            base = (i * GB + j) * 84
            for k8 in range(8):
                lhsT2[k8 * 13:k8 * 13 + 13,
                      base + 6 * j:base + 6 * j + 6] = W2[i]

    G1 = np.kron(np.eye(2 * GB, dtype=np.float32),
                 np.ones((3, 3), np.float32) / 3.0)  # [84, 84]

    cst = np.zeros((6, 9), np.float32)
    for i in range(N_LAYERS):
        cst[:, i] = 16.0 * b2[i]
        cst[:, 3 + i] = gw[i]
        cst[:, 6 + i] = gb[i]
    cst = np.tile(cst, (GB, 1))  # [84, 9]
    return lhsT1, lhsT2, G1, cst


def _build_nc(pp=PP):
    import concourse.bass as bass
    import concourse.bacc as bacc
    import concourse.mybir as mybir
    from concourse import tile
    from concourse.tile_rust import add_dep_helper as _adh

    def add_dep_helper(a, b, reason=""):
        _adh(a.ins, b.ins, reason=reason)

    bass_ds = bass.ds
    F32 = mybir.dt.float32
    F32R = mybir.dt.float32r
    AF = mybir.ActivationFunctionType
    OP = mybir.AluOpType

    MD = F32R if USE_F32R else F32

    import time as _time
    print(f"[kernel] build start {_time.time():.1f}", flush=True)
    nc = bacc.Bacc("TRN2", target_bir_lowering=False)
    x_e = nc.declare_dram_parameter("x", [128, pp], MD, isOutput=False)
    l1_e = nc.declare_dram_parameter("lhsT1", [128, N_LAYERS * 104], MD,
                                     isOutput=False)
    l2_e = nc.declare_dram_parameter("lhsT2", [104, N_LAYERS * GB * 84], MD,
                                     isOutput=False)
    g1_e = nc.declare_dram_parameter("g1", [SROWS, SROWS], F32, isOutput=False)
    cst_e = nc.declare_dram_parameter("cst", [SROWS, 9], F32, isOutput=False)
    out_e = nc.declare_dram_parameter("out", [SROWS, (pp // (GB * T)) * T], F32,
                                      isOutput=True)

    GBT = GB * T
    nb = pp // GBT
    if nb == NB:
        groups = GROUPS
    else:
        groups = [tuple(range(nb))]

    with tile.TileContext(nc) as tc:
        with tc.tile_pool(name="w", bufs=1) as wp, \
             tc.tile_pool(name="f", bufs=3) as fp, \
             tc.tile_pool(name="e", bufs=3) as ep, \
             tc.tile_pool(name="h", bufs=6) as hp, \
             tc.tile_pool(name="g", bufs=3) as gp, \
             tc.tile_pool(name="z", bufs=3, space="PSUM") as zp, \
             tc.tile_pool(name="s", bufs=2, space="PSUM") as sp, \
             tc.tile_pool(name="d", bufs=4, space="DRAM") as dp:
            l1 = wp.tile([128, N_LAYERS * 104], MD)
            l2 = wp.tile([104, N_LAYERS * GB * 84], MD)
            g1 = wp.tile([SROWS, SROWS], F32)
            cst = wp.tile([SROWS, 9], F32)
            eps = wp.tile([128, 1], F32)
            nc.sync.dma_start(out=l1[:], in_=l1_e[:])
            nc.sync.dma_start(out=l2[:], in_=l2_e[:])
            nc.sync.dma_start(out=g1[:], in_=g1_e[:])
            nc.sync.dma_start(out=cst[:], in_=cst_e[:])
            nc.gpsimd.memset(eps[:], EPS)

            Fs, Es = {}, {}
            eadd, mm1_last = {}, {}

            def preamble(b):
                F = fp.tile([128, GBT], MD, tag="F")
                nc.sync.dma_start(out=F[:], in_=x_e[:, bass_ds(b * GBT, GBT)])
                E = ep.tile([SROWS, T], F32, tag="E")
                nc.vector.memset(E[:], 1.0)
                Fs[b], Es[b] = F, E

            def unit(b, i):
                F, E = Fs[b], Es[b]
                refr = []
                if i > 0:
                    # Partition-crossing APs only lower correctly on the
                    # DRAM side, so bounce E through a DRAM scratch: the
                    # (j c)->(c j) partition mix happens in flat DRAM.
                    Ed = dp.tile([SROWS, T], F32, tag="Ed")
                    hop1 = nc.gpsimd.dma_start(out=Ed[:], in_=E[:])
                    add_dep_helper(hop1, eadd[b], reason="refresh RAW E")
                    esrc = Ed[:].rearrange("(j c) t -> c j t", j=GB, c=6)
                    for h in range(2):
                        dst = F[64 * h:64 * h + 6, :].rearrange(
                            "c (j t) -> c j t", j=GB, t=T)
                        d = nc.gpsimd.dma_start(out=dst, in_=esrc)
                        add_dep_helper(d, hop1, reason="refresh RAW Edram")
                        add_dep_helper(d, mm1_last[b], reason="refresh WAR F")
                        refr.append(d)
                stage = sp.tile([SROWS, T], F32, tag="stage")
                for j in range(GB):
                    Z = zp.tile([128, 2 * T], F32, tag="Z")
                    m1a = nc.tensor.matmul(Z[0:104, 0:T],
                                     l1[0:64, i * 104:(i + 1) * 104],
                                     F[0:64, bass_ds(j * T, T)],
                                     start=True, stop=True)
                    if j == 0:
                        for d in refr:
                            add_dep_helper(m1a, d, reason="MM1 RAW refresh")
                    m1b = nc.tensor.matmul(Z[0:104, T:2 * T],
                                     l1[64:128, i * 104:(i + 1) * 104],
                                     F[64:128, bass_ds(j * T, T)],
                                     start=True, stop=True)
                    mm1_last[b] = m1b
                    H = hp.tile([104, 2 * T], MD, tag="H")
                    nc.scalar.activation(H[:, 0:C1], Z[0:104, 0:C1],
                                         AF.Prelu, bias=0.0, scale=1.0,
                                         alpha=SLOPE)
                    # DVE reads PSUM only once per op: copy to SBUF,
                    # then lrelu = (x*0.2) max x.
                    tmp = hp.tile([104, 2 * T - C1], F32, tag="tmp")
                    nc.vector.tensor_copy(tmp[:], Z[0:104, C1:2 * T])
                    nc.vector.scalar_tensor_tensor(
                        H[:, C1:2 * T], tmp[:], SLOPE, tmp[:],
                        OP.mult, OP.max)
                    w2 = l2[:, (i * GB + j) * 84:(i * GB + j + 1) * 84]
                    nc.tensor.matmul(stage[:], w2, H[:, 0:T],
                                     start=(j == 0), stop=False)
                    nc.tensor.matmul(stage[:], w2, H[:, T:2 * T],
                                     start=False, stop=(j == GB - 1))

                # ---- batched GroupNorm over [84, T], fp32 ----
                bias = cst[0:SROWS, i:i + 1]
                sq = gp.tile([SROWS, T], F32, tag="sq")
                nc.scalar.activation(sq[:], stage[:], AF.Square, bias=bias,
                                     scale=1.0)
                st = gp.tile([SROWS, T], F32, tag="st")
                nc.vector.tensor_scalar(st[:], stage[:], bias, None, OP.add)
                # mu/m2 borrow a Z-pool slot (same shape) so two batches'
                # GN chains never serialize on a dedicated single buffer.
                mum2 = zp.tile([128, 2 * T], F32, tag="Z")
                nc.tensor.matmul(mum2[0:SROWS, 0:T], g1[:], st[:],
                                 start=True, stop=True)
                nc.tensor.matmul(mum2[0:SROWS, T:2 * T], g1[:], sq[:],
                                 start=True, stop=True)
                mu = mum2[0:SROWS, 0:T]
                m2 = mum2[0:SROWS, T:2 * T]
                mus2 = gp.tile([SROWS, T], F32, tag="mus2")
                nc.scalar.activation(mus2[:], mu, AF.Square)
                vpe = gp.tile([SROWS, T], F32, tag="vpe")
                nc.vector.scalar_tensor_tensor(
                    vpe[:], mus2[:], -1.0, m2, OP.mult, OP.add)
                rstd = gp.tile([SROWS, T], F32, tag="rstd")
                nc.scalar.activation(rstd[:], vpe[:],
                                     AF.Abs_reciprocal_sqrt,
                                     bias=eps[0:SROWS, 0:1], scale=1.0)
                y = gp.tile([SROWS, T], F32, tag="y")
                nc.vector.scalar_tensor_tensor(
                    y[:], mu, -1.0, st[:], OP.mult, OP.add)
                yr = gp.tile([SROWS, T], F32, tag="yr")
                nc.vector.tensor_mul(yr[:], y[:], rstd[:])
                upd = gp.tile([SROWS, T], F32, tag="upd")
                nc.scalar.activation(upd[:], yr[:], AF.Prelu,
                                     bias=cst[:, 6 + i:7 + i],
                                     scale=cst[:, 3 + i:4 + i], alpha=SLOPE)
                eadd[b] = nc.vector.tensor_add(E[:], E[:], upd[:])

            def outstore(b):
                E = Es[b]
                # gpsimd (SWDGE) keeps the out-store off the SP HWDGE
                # FIFO so it never head-of-line blocks the next loads.
                d = nc.gpsimd.dma_start(
                    out=out_e[:, bass_ds(b * T, T)], in_=E[:])
                add_dep_helper(d, eadd[b], reason="out RAW E")

            for grp in groups:
                for b in grp:
                    preamble(b)
                for i in range(N_LAYERS):
                    for b in grp:
                        unit(b, i)
                for b in grp:
                    outstore(b)

    print(f"[kernel] trace done {_time.time():.1f}", flush=True)
    nc.compile()
    print(f"[kernel] bacc compile done {_time.time():.1f}", flush=True)
    return nc


_RUNNER_CACHE = None


def get_runner():
    """Build (once) the jitted 8-core executable. Returns a dict with:
    fn(concat_inputs...) -> out jax arrays, in_names order, zero_outs."""
    global _RUNNER_CACHE
    if _RUNNER_CACHE is not None:
        return _RUNNER_CACHE
    import jax
    import numpy as _np
    from jax.sharding import Mesh, PartitionSpec
    from jax.experimental.shard_map import shard_map
    import concourse.mybir as mybir
    from concourse.bass2jax import (
        install_neuronx_cc_hook, _bass_exec_p, partition_id_tensor)

    nc = _build_nc()
    install_neuronx_cc_hook()
    partition_name = nc.partition_id_tensor.name if nc.partition_id_tensor else None
    in_names, out_names, out_avals, zero_outs = [], [], [], []
    for alloc in nc.m.functions[0].allocations:
        if not isinstance(alloc, mybir.MemoryLocationSet):
            continue
        name = alloc.memorylocations[0].name
        if alloc.kind == "ExternalInput":
            if name != partition_name:
                in_names.append(name)
        elif alloc.kind == "ExternalOutput":
            out_names.append(name)
            shape = tuple(alloc.tensor_shape)
            dtype = mybir.dt.np(alloc.dtype)
            out_avals.append(jax.core.ShapedArray(shape, dtype))
            zero_outs.append(_np.zeros(shape, dtype))
    n_params = len(in_names)
    all_in_names = in_names + out_names
    if partition_name is not None:
        all_in_names.append(partition_name)

    def _body(*args):
        operands = list(args)
        if partition_name is not None:
            operands.append(partition_id_tensor())
        return tuple(_bass_exec_p.bind(
            *operands,
            out_avals=tuple(out_avals),
            in_names=tuple(all_in_names),
            out_names=tuple(out_names),
            lowering_input_output_aliases=(),
            sim_require_finite=True,
            sim_require_nnan=True,
            nc=nc,
        ))

    devices = jax.devices()[:N_CORES]
    mesh = Mesh(_np.asarray(devices), ("core",))
    nin = n_params + len(out_names)
    sharded = jax.jit(shard_map(
        _body, mesh=mesh,
        in_specs=(PartitionSpec("core"),) * nin,
        out_specs=(PartitionSpec("core"),) * len(out_names),
        check_rep=False))
    _RUNNER_CACHE = {
        "fn": sharded, "in_names": in_names, "out_names": out_names,
        "zero_outs": zero_outs, "mesh": mesh,
    }
    return _RUNNER_CACHE


def run_cores(in_maps):
    """Run the 8-core kernel on a list of per-core input dicts."""
    import numpy as _np
    r = get_runner()
    concat_in = [
        _np.concatenate([in_maps[c][name] for c in range(N_CORES)], axis=0)
        for name in r["in_names"]
    ] + [_np.concatenate([z] * N_CORES, axis=0) for z in r["zero_outs"]]
    outs = r["fn"](*concat_in)
    res = []
    for c in range(N_CORES):
        d = {}
        for i, name in enumerate(r["out_names"]):
            full = _np.asarray(outs[i])
            rows = full.shape[0] // N_CORES
            d[name] = full[c * rows:(c + 1) * rows]
        res.append(d)
    return res


def make_in_maps(dist, atomtypes, W1, b1, W2, b2, gw, gb):
    dist = np.asarray(dist, np.float32)
    atomtypes = np.asarray(atomtypes, np.float32)
    B, N, _, _ = atomtypes.shape
    P = B * N
    assert P == N_CORES * PC

    lhsT1, lhsT2, G1, cst = _pack_weights(
        np.asarray(W1, np.float32), np.asarray(b1, np.float32),
        np.asarray(W2, np.float32), np.asarray(b2, np.float32),
        np.asarray(gw, np.float32), np.asarray(gb, np.float32))

    # X [128, PP] per core, rows matching F exactly: per half h:
    # 64h+0..5 emb=1.0, 64h+6 ones=1.0, 64h+7 pad, 64h+8+6k+c atom
    # (neighbors 8h..8h+7), 64h+56+k dist.
    atom = atomtypes.reshape(P, K, D)
    dst = dist.reshape(P, K)
    x = np.zeros((P, 128), np.float32)
    for h in range(2):
        x[:, 64 * h:64 * h + 7] = 1.0
        x[:, 64 * h + 8:64 * h + 56] = atom[:, 8 * h:8 * h + 8, :].reshape(P, 48)
        x[:, 64 * h + 56:64 * h + 64] = dst[:, 8 * h:8 * h + 8]

    in_maps = []
    for c in range(N_CORES):
        sl = slice(c * PC, (c + 1) * PC)
        xa = np.zeros((PP, 128), np.float32)
        xa[:PC] = x[sl]
        in_maps.append({
            "x": np.ascontiguousarray(xa.T),
            "lhsT1": lhsT1, "lhsT2": lhsT2, "g1": G1, "cst": cst,
        })
    return in_maps, (B, N)


def _unscramble(o):
    """[84, NB*T] per core -> [PP, 6] points: row 6j+c col b*T+t is point
    b*GB*T + j*T + t channel c."""
    o = o.reshape(GB, 6, NB, T)            # [j, c, b, t]
    return o.transpose(2, 0, 3, 1).reshape(PP, 6)


def kernel(dist, atomtypes, W1, b1, W2, b2, gw, gb):
    in_maps, (B, N) = make_in_maps(dist, atomtypes, W1, b1, W2, b2, gw, gb)
    res = run_cores(in_maps)
    outs = [_unscramble(res[c]["out"])[:PC] for c in range(N_CORES)]
    return np.concatenate(outs, axis=0).reshape(B, N, D).astype(np.float32)


if __name__ == "__main__":
    rng = np.random.default_rng(0)
    inputs = {
        "dist": rng.random((4, 100000, 16, 1), dtype=np.float32),
        "atomtypes": rng.random((4, 100000, 16, 6), dtype=np.float32),
        "W1": rng.random((3, 13, 13), dtype=np.float32) - 0.5,
        "b1": rng.random((3, 13), dtype=np.float32) - 0.5,
        "W2": rng.random((3, 13, 6), dtype=np.float32) - 0.5,
        "b2": rng.random((3, 6), dtype=np.float32) - 0.5,
        "gw": np.ones((3, 6), np.float32),
        "gb": np.zeros((3, 6), np.float32),
    }
    out = kernel(**inputs)
    print(out.shape, out.dtype)


# revision 27
# speedup vs baseline: 1.5507x; 1.4077x over previous
"""Trainium2 Bass kernel: 3-layer GNN message passing (atom embedding).

Data-parallel over the B*N=400000 point axis across 8 NeuronCores.

Layout (channels-on-partitions, fp32 data, float32r matmuls):
  F [128, GB*T] fp32 per batch of GB=14 point-tiles; each 64-partition
  half h holds neighbors 8h..8h+7 of every point:
    rows 64h+0..5   emb (refreshed per layer from E)
    row  64h+6      const 1.0 (folds b1 into matmul1)
    rows 64h+8+6k+c atom   (k=0..7 within half)
    rows 64h+56+k   dist
  matmul1 per tile: two K=64 matmuls (one per half, identical lhsT1
  content replicated at partitions 0-63 / 64-127) -> Z [104, 2T] PSUM.
  LeakyReLU evacuation Z->H split between ScalarE (Prelu, cols 0:C1)
  and VectorE (copy to SBUF then (x*0.2) max x; DVE may read PSUM only
  once per op).
  matmul2: per-tile lhsT2 [104,84] (W2 at output cols 6j) accumulates
  all GB tiles' messages directly into one batched PSUM stage [84, T];
  the +16*b2 bias is folded into the two PSUM->SBUF reads.
  GroupNorm batched over [84, T].

  Precision: GroupNorm's var=m2-mu^2 sits at the eps=1e-5 threshold on
  this data, amplifying msg noise ~300x — ANY 16-bit quantization of
  inputs/weights/h fails the 2e-2 tolerance (bf16 -> rel 0.3, fp16 ->
  0.23, measured).  So all tensors stay fp32; the 1176 main matmuls run
  as float32r (bitcast views: full-rate rows vs 4 cycles/row for plain
  fp32, and self-loading — no per-matmul LDWEIGHTS instruction).  The
  42 GN stats matmuls stay plain fp32: float32r rounding in m2 would
  swamp var at the eps scale.

  Batches are emitted software-pipelined in groups of 2-3 (units
  (b0,i),(b1,i),(b0,i+1)...) so the per-layer serial GN chain of one
  batch overlaps the tile crunch of another instead of stalling PE.
  ~7 DMAs per batch (batched loads, one rearranged-AP scatter per half
  for the emb refresh, gpsimd out-store off the SP HWDGE FIFO).
"""
import sys

sys.path.insert(0, "/opt/trn_rl_repo")

import numpy as np

D = 6
K = 16
N_LAYERS = 3
C_IN = 13
EPS = 1e-5
SLOPE = 0.2

N_CORES = 8
T = 512            # points per tile (PSUM bank = 512 fp32)
PC = 50000         # points per core
NT = 98            # tiles per core
PP = NT * T        # padded points per core = 50176
GB = 14            # point-tiles per groupnorm batch (84 partitions)
SROWS = 6 * GB     # 84
C1 = 736           # ScalarE/VectorE split of the Z->H evacuation
NB = NT // GB      # 7 batches
# batch groups for software pipelining (2-3 batches in flight)
GROUPS = [(0, 1), (2, 3), (4, 5, 6)]
USE_F32R = False


def _pack_weights(W1, b1, W2, b2, gw, gb):
    """Build packed lhsT / const tensors (host side, a few KB)."""
    # lhsT1 [128, 3*104]: rows 0-63 = half template, 64-127 copy.
    half = np.zeros((64, N_LAYERS * 104), np.float32)
    for i in range(N_LAYERS):
        for k8 in range(8):
            cols = slice(i * 104 + k8 * 13, i * 104 + k8 * 13 + 13)
            half[0:6, cols] = W1[i, 0:6, :]          # emb rows
            half[6, cols] = b1[i]                    # ones row -> bias
            half[8 + k8 * 6:8 + k8 * 6 + 6, cols] = W1[i, 6:12, :]
            half[56 + k8, cols] = W1[i, 12, :]       # dist row
    lhsT1 = np.concatenate([half, half], axis=0)     # [128, 312]

    # lhsT2 [104, 3*GB*84]: variant (i, j) puts W2[i] at output cols
    # 6j..6j+6 for every neighbor block.
    lhsT2 = np.zeros((104, N_LAYERS * GB * 84), np.float32)
    for i in range(N_LAYERS):
        for j in range(GB):
            base = (i * GB + j) * 84
            for k8 in range(8):
                lhsT2[k8 * 13:k8 * 13 + 13,
                      base + 6 * j:base + 6 * j + 6] = W2[i]

    G1 = np.kron(np.eye(2 * GB, dtype=np.float32),
                 np.ones((3, 3), np.float32) / 3.0)  # [84, 84]

    cst = np.zeros((6, 9), np.float32)
    for i in range(N_LAYERS):
        cst[:, i] = 16.0 * b2[i]
        cst[:, 3 + i] = gw[i]
        cst[:, 6 + i] = gb[i]
    cst = np.tile(cst, (GB, 1))  # [84, 9]
    return lhsT1, lhsT2, G1, cst


def _build_nc(pp=PP, repeat=1):
    import concourse.bass as bass
    import concourse.bacc as bacc
    import concourse.mybir as mybir
    from concourse import tile
    from concourse.tile_rust import add_dep_helper as _adh

    def add_dep_helper(a, b, reason=""):
        _adh(a.ins, b.ins, reason=reason)

    bass_ds = bass.ds
    F32 = mybir.dt.float32
    F32R = mybir.dt.float32r
    AF = mybir.ActivationFunctionType
    OP = mybir.AluOpType

    MD = F32R if USE_F32R else F32

    import time as _time
    print(f"[kernel] build start {_time.time():.1f}", flush=True)
    nc = bacc.Bacc("TRN2", target_bir_lowering=False)
    x_e = nc.declare_dram_parameter("x", [128, pp], MD, isOutput=False)
    l1_e = nc.declare_dram_parameter("lhsT1", [128, N_LAYERS * 104], MD,
                                     isOutput=False)
    l2_e = nc.declare_dram_parameter("lhsT2", [104, N_LAYERS * GB * 84], MD,
                                     isOutput=False)
    g1_e = nc.declare_dram_parameter("g1", [SROWS, SROWS], F32, isOutput=False)
    cst_e = nc.declare_dram_parameter("cst", [SROWS, 9], F32, isOutput=False)
    out_e = nc.declare_dram_parameter("out", [SROWS, (pp // (GB * T)) * T], F32,
                                      isOutput=True)

    GBT = GB * T
    nb = pp // GBT
    if nb == NB:
        groups = GROUPS
    else:
        groups = [tuple(range(nb))]

    with tile.TileContext(nc) as tc:
        with tc.tile_pool(name="w", bufs=1) as wp, \
             tc.tile_pool(name="f", bufs=3) as fp, \
             tc.tile_pool(name="e", bufs=3) as ep, \
             tc.tile_pool(name="h", bufs=6) as hp, \
             tc.tile_pool(name="g", bufs=3) as gp, \
             tc.tile_pool(name="z", bufs=3, space="PSUM") as zp, \
             tc.tile_pool(name="s", bufs=2, space="PSUM") as sp, \
             tc.tile_pool(name="d", bufs=4, space="DRAM") as dp:
            l1 = wp.tile([128, N_LAYERS * 104], MD)
            l2 = wp.tile([104, N_LAYERS * GB * 84], MD)
            g1 = wp.tile([SROWS, SROWS], F32)
            cst = wp.tile([SROWS, 9], F32)
            eps = wp.tile([128, 1], F32)
            nc.sync.dma_start(out=l1[:], in_=l1_e[:])
            nc.sync.dma_start(out=l2[:], in_=l2_e[:])
            nc.sync.dma_start(out=g1[:], in_=g1_e[:])
            nc.sync.dma_start(out=cst[:], in_=cst_e[:])
            nc.gpsimd.memset(eps[:], EPS)

            Fs, Es = {}, {}
            eadd, mm1_last = {}, {}

            def preamble(b):
                F = fp.tile([128, GBT], MD, tag="F")
                nc.sync.dma_start(out=F[:], in_=x_e[:, bass_ds(b * GBT, GBT)])
                E = ep.tile([SROWS, T], F32, tag="E")
                nc.vector.memset(E[:], 1.0)
                Fs[b], Es[b] = F, E

            def unit(b, i):
                F, E = Fs[b], Es[b]
                refr = []
                if i > 0:
                    # Partition-crossing APs only lower correctly on the
                    # DRAM side, so bounce E through a DRAM scratch: the
                    # (j c)->(c j) partition mix happens in flat DRAM.
                    Ed = dp.tile([SROWS, T], F32, tag="Ed")
                    hop1 = nc.gpsimd.dma_start(out=Ed[:], in_=E[:])
                    add_dep_helper(hop1, eadd[b], reason="refresh RAW E")
                    esrc = Ed[:].rearrange("(j c) t -> c j t", j=GB, c=6)
                    for h in range(2):
                        dst = F[64 * h:64 * h + 6, :].rearrange(
                            "c (j t) -> c j t", j=GB, t=T)
                        d = nc.gpsimd.dma_start(out=dst, in_=esrc)
                        add_dep_helper(d, hop1, reason="refresh RAW Edram")
                        add_dep_helper(d, mm1_last[b], reason="refresh WAR F")
                        refr.append(d)
                stage = sp.tile([SROWS, T], F32, tag="stage")
                for j in range(GB):
                    Z = zp.tile([128, 2 * T], F32, tag="Z")
                    m1a = nc.tensor.matmul(Z[0:104, 0:T],
                                     l1[0:64, i * 104:(i + 1) * 104],
                                     F[0:64, bass_ds(j * T, T)],
                                     start=True, stop=True)
                    if j == 0:
                        for d in refr:
                            add_dep_helper(m1a, d, reason="MM1 RAW refresh")
                    m1b = nc.tensor.matmul(Z[0:104, T:2 * T],
                                     l1[64:128, i * 104:(i + 1) * 104],
                                     F[64:128, bass_ds(j * T, T)],
                                     start=True, stop=True)
                    mm1_last[b] = m1b
                    H = hp.tile([104, 2 * T], MD, tag="H")
                    nc.scalar.activation(H[:, 0:C1], Z[0:104, 0:C1],
                                         AF.Prelu, bias=0.0, scale=1.0,
                                         alpha=SLOPE)
                    # DVE reads PSUM only once per op: copy to SBUF,
                    # then lrelu = (x*0.2) max x.
                    tmp = hp.tile([104, 2 * T - C1], F32, tag="tmp")
                    nc.vector.tensor_copy(tmp[:], Z[0:104, C1:2 * T])
                    nc.vector.scalar_tensor_tensor(
                        H[:, C1:2 * T], tmp[:], SLOPE, tmp[:],
                        OP.mult, OP.max)
                    w2 = l2[:, (i * GB + j) * 84:(i * GB + j + 1) * 84]
                    nc.tensor.matmul(stage[:], w2, H[:, 0:T],
                                     start=(j == 0), stop=False)
                    nc.tensor.matmul(stage[:], w2, H[:, T:2 * T],
                                     start=False, stop=(j == GB - 1))

                # ---- batched GroupNorm over [84, T], fp32 ----
                bias = cst[0:SROWS, i:i + 1]
                sq = gp.tile([SROWS, T], F32, tag="sq")
                nc.scalar.activation(sq[:], stage[:], AF.Square, bias=bias,
                                     scale=1.0)
                st = gp.tile([SROWS, T], F32, tag="st")
                nc.vector.tensor_scalar(st[:], stage[:], bias, None, OP.add)
                # mu/m2 borrow a Z-pool slot (same shape) so two batches'
                # GN chains never serialize on a dedicated single buffer.
                mum2 = zp.tile([128, 2 * T], F32, tag="Z")
                nc.tensor.matmul(mum2[0:SROWS, 0:T], g1[:], st[:],
                                 start=True, stop=True)
                nc.tensor.matmul(mum2[0:SROWS, T:2 * T], g1[:], sq[:],
                                 start=True, stop=True)
                mu = mum2[0:SROWS, 0:T]
                m2 = mum2[0:SROWS, T:2 * T]
                mus2 = gp.tile([SROWS, T], F32, tag="mus2")
                nc.scalar.activation(mus2[:], mu, AF.Square)
                vpe = gp.tile([SROWS, T], F32, tag="vpe")
                nc.vector.scalar_tensor_tensor(
                    vpe[:], mus2[:], -1.0, m2, OP.mult, OP.add)
                rstd = gp.tile([SROWS, T], F32, tag="rstd")
                nc.scalar.activation(rstd[:], vpe[:],
                                     AF.Abs_reciprocal_sqrt,
                                     bias=eps[0:SROWS, 0:1], scale=1.0)
                y = gp.tile([SROWS, T], F32, tag="y")
                nc.vector.scalar_tensor_tensor(
                    y[:], mu, -1.0, st[:], OP.mult, OP.add)
                yr = gp.tile([SROWS, T], F32, tag="yr")
                nc.vector.tensor_mul(yr[:], y[:], rstd[:])
                upd = gp.tile([SROWS, T], F32, tag="upd")
                nc.scalar.activation(upd[:], yr[:], AF.Prelu,
                                     bias=cst[:, 6 + i:7 + i],
                                     scale=cst[:, 3 + i:4 + i], alpha=SLOPE)
                eadd[b] = nc.vector.tensor_add(E[:], E[:], upd[:])

            def outstore(b):
                E = Es[b]
                # gpsimd (SWDGE) keeps the out-store off the SP HWDGE
                # FIFO so it never head-of-line blocks the next loads.
                d = nc.gpsimd.dma_start(
                    out=out_e[:, bass_ds(b * T, T)], in_=E[:])
                add_dep_helper(d, eadd[b], reason="out RAW E")

            def whole():
                for grp in groups:
                    for b in grp:
                        preamble(b)
                    for i in range(N_LAYERS):
                        for b in grp:
                            unit(b, i)
                    for b in grp:
                        outstore(b)

            if repeat > 1:
                # In-NEFF repeat loop: amortizes the ~2 ms per-dispatch
                # axon/PJRT overhead across `repeat` full executions so
                # the marginal kernel time is measurable.
                with tc.For_i(0, repeat, 1):
                    whole()
            else:
                whole()

    print(f"[kernel] trace done {_time.time():.1f}", flush=True)
    nc.compile()
    print(f"[kernel] bacc compile done {_time.time():.1f}", flush=True)
    return nc


_RUNNER_CACHE = {}


def get_runner(repeat=1):
    """Build (once per repeat) the jitted 8-core executable. Returns a dict:
    fn(concat_inputs...) -> out jax arrays, in_names order, zero_outs."""
    global _RUNNER_CACHE
    if repeat in _RUNNER_CACHE:
        return _RUNNER_CACHE[repeat]
    import jax
    import numpy as _np
    from jax.sharding import Mesh, PartitionSpec
    from jax.experimental.shard_map import shard_map
    import concourse.mybir as mybir
    from concourse.bass2jax import (
        install_neuronx_cc_hook, _bass_exec_p, partition_id_tensor)

    nc = _build_nc(repeat=repeat)
    install_neuronx_cc_hook()
    partition_name = nc.partition_id_tensor.name if nc.partition_id_tensor else None
    in_names, out_names, out_avals, zero_outs = [], [], [], []
    for alloc in nc.m.functions[0].allocations:
        if not isinstance(alloc, mybir.MemoryLocationSet):
            continue
        name = alloc.memorylocations[0].name
        if alloc.kind == "ExternalInput":
            if name != partition_name:
                in_names.append(name)
        elif alloc.kind == "ExternalOutput":
            out_names.append(name)
            shape = tuple(alloc.tensor_shape)
            dtype = mybir.dt.np(alloc.dtype)
            out_avals.append(jax.core.ShapedArray(shape, dtype))
            zero_outs.append(_np.zeros(shape, dtype))
    n_params = len(in_names)
    all_in_names = in_names + out_names
    if partition_name is not None:
        all_in_names.append(partition_name)

    def _body(*args):
        operands = list(args)
        if partition_name is not None:
            operands.append(partition_id_tensor())
        return tuple(_bass_exec_p.bind(
            *operands,
            out_avals=tuple(out_avals),
            in_names=tuple(all_in_names),
            out_names=tuple(out_names),
            lowering_input_output_aliases=(),
            sim_require_finite=True,
            sim_require_nnan=True,
            nc=nc,
        ))

    devices = jax.devices()[:N_CORES]
    mesh = Mesh(_np.asarray(devices), ("core",))
    nin = n_params + len(out_names)
    sharded = jax.jit(shard_map(
        _body, mesh=mesh,
        in_specs=(PartitionSpec("core"),) * nin,
        out_specs=(PartitionSpec("core"),) * len(out_names),
        check_rep=False))
    _RUNNER_CACHE[repeat] = {
        "fn": sharded, "in_names": in_names, "out_names": out_names,
        "zero_outs": zero_outs, "mesh": mesh,
    }
    return _RUNNER_CACHE[repeat]


def run_cores(in_maps):
    """Run the 8-core kernel on a list of per-core input dicts."""
    import numpy as _np
    r = get_runner()
    concat_in = [
        _np.concatenate([in_maps[c][name] for c in range(N_CORES)], axis=0)
        for name in r["in_names"]
    ] + [_np.concatenate([z] * N_CORES, axis=0) for z in r["zero_outs"]]
    outs = r["fn"](*concat_in)
    res = []
    for c in range(N_CORES):
        d = {}
        for i, name in enumerate(r["out_names"]):
            full = _np.asarray(outs[i])
            rows = full.shape[0] // N_CORES
            d[name] = full[c * rows:(c + 1) * rows]
        res.append(d)
    return res


def make_in_maps(dist, atomtypes, W1, b1, W2, b2, gw, gb):
    dist = np.asarray(dist, np.float32)
    atomtypes = np.asarray(atomtypes, np.float32)
    B, N, _, _ = atomtypes.shape
    P = B * N
    assert P == N_CORES * PC

    lhsT1, lhsT2, G1, cst = _pack_weights(
        np.asarray(W1, np.float32), np.asarray(b1, np.float32),
        np.asarray(W2, np.float32), np.asarray(b2, np.float32),
        np.asarray(gw, np.float32), np.asarray(gb, np.float32))

    # X [128, PP] per core, rows matching F exactly: per half h:
    # 64h+0..5 emb=1.0, 64h+6 ones=1.0, 64h+7 pad, 64h+8+6k+c atom
    # (neighbors 8h..8h+7), 64h+56+k dist.
    atom = atomtypes.reshape(P, K, D)
    dst = dist.reshape(P, K)
    x = np.zeros((P, 128), np.float32)
    for h in range(2):
        x[:, 64 * h:64 * h + 7] = 1.0
        x[:, 64 * h + 8:64 * h + 56] = atom[:, 8 * h:8 * h + 8, :].reshape(P, 48)
        x[:, 64 * h + 56:64 * h + 64] = dst[:, 8 * h:8 * h + 8]

    in_maps = []
    for c in range(N_CORES):
        sl = slice(c * PC, (c + 1) * PC)
        xa = np.zeros((PP, 128), np.float32)
        xa[:PC] = x[sl]
        in_maps.append({
            "x": np.ascontiguousarray(xa.T),
            "lhsT1": lhsT1, "lhsT2": lhsT2, "g1": G1, "cst": cst,
        })
    return in_maps, (B, N)


def _unscramble(o):
    """[84, NB*T] per core -> [PP, 6] points: row 6j+c col b*T+t is point
    b*GB*T + j*T + t channel c."""
    o = o.reshape(GB, 6, NB, T)            # [j, c, b, t]
    return o.transpose(2, 0, 3, 1).reshape(PP, 6)


def kernel(dist, atomtypes, W1, b1, W2, b2, gw, gb):
    in_maps, (B, N) = make_in_maps(dist, atomtypes, W1, b1, W2, b2, gw, gb)
    res = run_cores(in_maps)
    outs = [_unscramble(res[c]["out"])[:PC] for c in range(N_CORES)]
    return np.concatenate(outs, axis=0).reshape(B, N, D).astype(np.float32)


if __name__ == "__main__":
    rng = np.random.default_rng(0)
    inputs = {
        "dist": rng.random((4, 100000, 16, 1), dtype=np.float32),
        "atomtypes": rng.random((4, 100000, 16, 6), dtype=np.float32),
        "W1": rng.random((3, 13, 13), dtype=np.float32) - 0.5,
        "b1": rng.random((3, 13), dtype=np.float32) - 0.5,
        "W2": rng.random((3, 13, 6), dtype=np.float32) - 0.5,
        "b2": rng.random((3, 6), dtype=np.float32) - 0.5,
        "gw": np.ones((3, 6), np.float32),
        "gb": np.zeros((3, 6), np.float32),
    }
    out = kernel(**inputs)
    print(out.shape, out.dtype)


# revision 28
# speedup vs baseline: 2.0043x; 1.2925x over previous
"""Trainium2 Bass kernel: 3-layer GNN message passing (atom embedding).

Data-parallel over the B*N=400000 point axis across 8 NeuronCores.

Layout (channels-on-partitions, fp32 data, float32r matmuls):
  F [128, GB*T] fp32 per batch of GB=14 point-tiles; each 64-partition
  half h holds neighbors 8h..8h+7 of every point:
    rows 64h+0..5   emb (refreshed per layer from E)
    row  64h+6      const 1.0 (folds b1 into matmul1)
    rows 64h+8+6k+c atom   (k=0..7 within half)
    rows 64h+56+k   dist
  matmul1 per tile: two K=64 matmuls (one per half, identical lhsT1
  content replicated at partitions 0-63 / 64-127) -> Z [104, 2T] PSUM.
  LeakyReLU evacuation Z->H split between ScalarE (Prelu, cols 0:C1)
  and VectorE (copy to SBUF then (x*0.2) max x; DVE may read PSUM only
  once per op).
  matmul2: per-tile lhsT2 [104,84] (W2 at output cols 6j) accumulates
  all GB tiles' messages directly into one batched PSUM stage [84, T];
  the +16*b2 bias is folded into the two PSUM->SBUF reads.
  GroupNorm batched over [84, T].

  Precision: GroupNorm's var=m2-mu^2 sits at the eps=1e-5 threshold on
  this data, amplifying msg noise ~300x — ANY 16-bit quantization of
  inputs/weights/h fails the 2e-2 tolerance (bf16 -> rel 0.3, fp16 ->
  0.23, measured).  So all tensors stay fp32; the 1176 main matmuls run
  as float32r (bitcast views: full-rate rows vs 4 cycles/row for plain
  fp32, and self-loading — no per-matmul LDWEIGHTS instruction).  The
  42 GN stats matmuls stay plain fp32: float32r rounding in m2 would
  swamp var at the eps scale.

  Batches are emitted software-pipelined in groups of 2-3 (units
  (b0,i),(b1,i),(b0,i+1)...) so the per-layer serial GN chain of one
  batch overlaps the tile crunch of another instead of stalling PE.
  ~7 DMAs per batch (batched loads, one rearranged-AP scatter per half
  for the emb refresh, gpsimd out-store off the SP HWDGE FIFO).
"""
import sys

sys.path.insert(0, "/opt/trn_rl_repo")

import numpy as np

D = 6
K = 16
N_LAYERS = 3
C_IN = 13
EPS = 1e-5
SLOPE = 0.2

N_CORES = 8
T = 512            # points per tile (PSUM bank = 512 fp32)
PC = 50000         # points per core
NT = 98            # tiles per core
PP = NT * T        # padded points per core = 50176
GB = 14            # point-tiles per groupnorm batch (84 partitions)
SROWS = 6 * GB     # 84
C1 = 736           # ScalarE/VectorE split of the Z->H evacuation
NB = NT // GB      # 7 batches
# batch groups for software pipelining (2-3 batches in flight)
GROUPS = [(0, 1), (2, 3), (4, 5, 6)]
USE_F32R = False


def _pack_weights(W1, b1, W2, b2, gw, gb):
    """Build packed lhsT / const tensors (host side, a few KB)."""
    # lhsT1 [128, 3*104]: rows 0-63 = half template, 64-127 copy.
    half = np.zeros((64, N_LAYERS * 104), np.float32)
    for i in range(N_LAYERS):
        for k8 in range(8):
            cols = slice(i * 104 + k8 * 13, i * 104 + k8 * 13 + 13)
            half[0:6, cols] = W1[i, 0:6, :]          # emb rows
            half[6, cols] = b1[i]                    # ones row -> bias
            half[8 + k8 * 6:8 + k8 * 6 + 6, cols] = W1[i, 6:12, :]
            half[56 + k8, cols] = W1[i, 12, :]       # dist row
    lhsT1 = np.concatenate([half, half], axis=0)     # [128, 312]

    # lhsT2 [104, 3*GB*84]: variant (i, j) puts W2[i] at output cols
    # 6j..6j+6 for every neighbor block.
    lhsT2 = np.zeros((104, N_LAYERS * GB * 84), np.float32)
    for i in range(N_LAYERS):
        for j in range(GB):
            base = (i * GB + j) * 84
            for k8 in range(8):
                lhsT2[k8 * 13:k8 * 13 + 13,
                      base + 6 * j:base + 6 * j + 6] = W2[i]

    G1 = np.kron(np.eye(2 * GB, dtype=np.float32),
                 np.ones((3, 3), np.float32) / 3.0)  # [84, 84]

    cst = np.zeros((6, 9), np.float32)
    for i in range(N_LAYERS):
        cst[:, i] = 16.0 * b2[i]
        cst[:, 3 + i] = gw[i]
        cst[:, 6 + i] = gb[i]
    cst = np.tile(cst, (GB, 1))  # [84, 9]
    return lhsT1, lhsT2, G1, cst


def _build_nc(pp=PP, repeat=1):
    import concourse.bass as bass
    import concourse.bacc as bacc
    import concourse.mybir as mybir
    from concourse import tile
    from concourse.tile_rust import add_dep_helper as _adh

    def add_dep_helper(a, b, reason=""):
        _adh(a.ins, b.ins, reason=reason)

    bass_ds = bass.ds
    F32 = mybir.dt.float32
    F32R = mybir.dt.float32r
    AF = mybir.ActivationFunctionType
    OP = mybir.AluOpType

    MD = F32R if USE_F32R else F32

    import time as _time
    print(f"[kernel] build start {_time.time():.1f}", flush=True)
    nc = bacc.Bacc("TRN2", target_bir_lowering=False)
    x_e = nc.declare_dram_parameter("x", [128, pp], MD, isOutput=False)
    l1_e = nc.declare_dram_parameter("lhsT1", [128, N_LAYERS * 104], MD,
                                     isOutput=False)
    l2_e = nc.declare_dram_parameter("lhsT2", [104, N_LAYERS * GB * 84], MD,
                                     isOutput=False)
    g1_e = nc.declare_dram_parameter("g1", [SROWS, SROWS], F32, isOutput=False)
    cst_e = nc.declare_dram_parameter("cst", [SROWS, 9], F32, isOutput=False)
    out_e = nc.declare_dram_parameter("out", [SROWS, (pp // (GB * T)) * T], F32,
                                      isOutput=True)

    GBT = GB * T
    nb = pp // GBT
    if nb == NB:
        groups = GROUPS
    else:
        groups = [tuple(range(nb))]

    with tile.TileContext(nc) as tc:
        with tc.tile_pool(name="w", bufs=1) as wp, \
             tc.tile_pool(name="f", bufs=3) as fp, \
             tc.tile_pool(name="e", bufs=3) as ep, \
             tc.tile_pool(name="h", bufs=8) as hp, \
             tc.tile_pool(name="g", bufs=3) as gp, \
             tc.tile_pool(name="z", bufs=3, space="PSUM") as zp, \
             tc.tile_pool(name="s", bufs=2, space="PSUM") as sp, \
             tc.tile_pool(name="d", bufs=4, space="DRAM") as dp:
            l1 = wp.tile([128, N_LAYERS * 104], MD)
            l2 = wp.tile([104, N_LAYERS * GB * 84], MD)
            g1 = wp.tile([SROWS, SROWS], F32)
            cst = wp.tile([SROWS, 9], F32)
            eps = wp.tile([128, 1], F32)
            nc.sync.dma_start(out=l1[:], in_=l1_e[:])
            nc.sync.dma_start(out=l2[:], in_=l2_e[:])
            nc.sync.dma_start(out=g1[:], in_=g1_e[:])
            nc.sync.dma_start(out=cst[:], in_=cst_e[:])
            nc.gpsimd.memset(eps[:], EPS)

            Fs, Es = {}, {}
            eadd, mm1_last = {}, {}

            def preamble(b):
                F = fp.tile([128, GBT], MD, tag="F")
                nc.sync.dma_start(out=F[:], in_=x_e[:, bass_ds(b * GBT, GBT)])
                E = ep.tile([SROWS, T], F32, tag="E")
                nc.vector.memset(E[:], 1.0)
                Fs[b], Es[b] = F, E

            def unit(b, i):
                F, E = Fs[b], Es[b]
                refr = []
                if i > 0:
                    # Partition-crossing APs only lower correctly on the
                    # DRAM side, so bounce E through a DRAM scratch: the
                    # (j c)->(c j) partition mix happens in flat DRAM.
                    Ed = dp.tile([SROWS, T], F32, tag="Ed")
                    hop1 = nc.gpsimd.dma_start(out=Ed[:], in_=E[:])
                    add_dep_helper(hop1, eadd[b], reason="refresh RAW E")
                    esrc = Ed[:].rearrange("(j c) t -> c j t", j=GB, c=6)
                    for h in range(2):
                        dst = F[64 * h:64 * h + 6, :].rearrange(
                            "c (j t) -> c j t", j=GB, t=T)
                        d = nc.gpsimd.dma_start(out=dst, in_=esrc)
                        add_dep_helper(d, hop1, reason="refresh RAW Edram")
                        add_dep_helper(d, mm1_last[b], reason="refresh WAR F")
                        refr.append(d)
                stage = sp.tile([SROWS, T], F32, tag="stage")
                for j in range(GB):
                    Z = zp.tile([128, 2 * T], F32, tag="Z")
                    m1a = nc.tensor.matmul(Z[0:104, 0:T],
                                     l1[0:64, i * 104:(i + 1) * 104],
                                     F[0:64, bass_ds(j * T, T)],
                                     start=True, stop=True)
                    if j == 0:
                        for d in refr:
                            add_dep_helper(m1a, d, reason="MM1 RAW refresh")
                    m1b = nc.tensor.matmul(Z[0:104, T:2 * T],
                                     l1[64:128, i * 104:(i + 1) * 104],
                                     F[64:128, bass_ds(j * T, T)],
                                     start=True, stop=True)
                    mm1_last[b] = m1b
                    H = hp.tile([104, 2 * T], MD, tag="H")
                    nc.scalar.activation(H[:, 0:C1], Z[0:104, 0:C1],
                                         AF.Prelu, bias=0.0, scale=1.0,
                                         alpha=SLOPE)
                    # DVE reads PSUM only once per op: copy to SBUF,
                    # then lrelu = (x*0.2) max x.
                    tmp = hp.tile([104, 2 * T - C1], F32, tag="tmp")
                    nc.vector.tensor_copy(tmp[:], Z[0:104, C1:2 * T])
                    nc.vector.scalar_tensor_tensor(
                        H[:, C1:2 * T], tmp[:], SLOPE, tmp[:],
                        OP.mult, OP.max)
                    w2 = l2[:, (i * GB + j) * 84:(i * GB + j + 1) * 84]
                    nc.tensor.matmul(stage[:], w2, H[:, 0:T],
                                     start=(j == 0), stop=False)
                    nc.tensor.matmul(stage[:], w2, H[:, T:2 * T],
                                     start=False, stop=(j == GB - 1))

                # ---- batched GroupNorm over [84, T], fp32 ----
                bias = cst[0:SROWS, i:i + 1]
                sq = gp.tile([SROWS, T], F32, tag="sq")
                nc.scalar.activation(sq[:], stage[:], AF.Square, bias=bias,
                                     scale=1.0)
                st = gp.tile([SROWS, T], F32, tag="st")
                nc.vector.tensor_scalar(st[:], stage[:], bias, None, OP.add)
                # mu/m2 borrow a Z-pool slot (same shape) so two batches'
                # GN chains never serialize on a dedicated single buffer.
                mum2 = zp.tile([128, 2 * T], F32, tag="Z")
                nc.tensor.matmul(mum2[0:SROWS, 0:T], g1[:], st[:],
                                 start=True, stop=True)
                nc.tensor.matmul(mum2[0:SROWS, T:2 * T], g1[:], sq[:],
                                 start=True, stop=True)
                mu = mum2[0:SROWS, 0:T]
                m2 = mum2[0:SROWS, T:2 * T]
                mus2 = gp.tile([SROWS, T], F32, tag="mus2")
                nc.scalar.activation(mus2[:], mu, AF.Square)
                vpe = gp.tile([SROWS, T], F32, tag="vpe")
                nc.vector.scalar_tensor_tensor(
                    vpe[:], mus2[:], -1.0, m2, OP.mult, OP.add)
                rstd = gp.tile([SROWS, T], F32, tag="rstd")
                nc.scalar.activation(rstd[:], vpe[:],
                                     AF.Abs_reciprocal_sqrt,
                                     bias=eps[0:SROWS, 0:1], scale=1.0)
                y = gp.tile([SROWS, T], F32, tag="y")
                nc.vector.scalar_tensor_tensor(
                    y[:], mu, -1.0, st[:], OP.mult, OP.add)
                yr = gp.tile([SROWS, T], F32, tag="yr")
                nc.vector.tensor_mul(yr[:], y[:], rstd[:])
                upd = gp.tile([SROWS, T], F32, tag="upd")
                nc.scalar.activation(upd[:], yr[:], AF.Prelu,
                                     bias=cst[:, 6 + i:7 + i],
                                     scale=cst[:, 3 + i:4 + i], alpha=SLOPE)
                eadd[b] = nc.vector.tensor_add(E[:], E[:], upd[:])

            def outstore(b):
                E = Es[b]
                # gpsimd (SWDGE) keeps the out-store off the SP HWDGE
                # FIFO so it never head-of-line blocks the next loads.
                d = nc.gpsimd.dma_start(
                    out=out_e[:, bass_ds(b * T, T)], in_=E[:])
                add_dep_helper(d, eadd[b], reason="out RAW E")

            def whole():
                for grp in groups:
                    for b in grp:
                        preamble(b)
                    for i in range(N_LAYERS):
                        for b in grp:
                            unit(b, i)
                    for b in grp:
                        outstore(b)

            if repeat > 1:
                # In-NEFF repeat loop: amortizes the ~2 ms per-dispatch
                # axon/PJRT overhead across `repeat` full executions so
                # the marginal kernel time is measurable.
                with tc.For_i(0, repeat, 1):
                    whole()
            else:
                whole()

    print(f"[kernel] trace done {_time.time():.1f}", flush=True)
    nc.compile()
    print(f"[kernel] bacc compile done {_time.time():.1f}", flush=True)
    return nc


_RUNNER_CACHE = {}


def get_runner(repeat=1):
    """Build (once per repeat) the jitted 8-core executable. Returns a dict:
    fn(concat_inputs...) -> out jax arrays, in_names order, zero_outs."""
    global _RUNNER_CACHE
    if repeat in _RUNNER_CACHE:
        return _RUNNER_CACHE[repeat]
    import jax
    import numpy as _np
    from jax.sharding import Mesh, PartitionSpec
    from jax.experimental.shard_map import shard_map
    import concourse.mybir as mybir
    from concourse.bass2jax import (
        install_neuronx_cc_hook, _bass_exec_p, partition_id_tensor)

    nc = _build_nc(repeat=repeat)
    install_neuronx_cc_hook()
    partition_name = nc.partition_id_tensor.name if nc.partition_id_tensor else None
    in_names, out_names, out_avals, zero_outs = [], [], [], []
    for alloc in nc.m.functions[0].allocations:
        if not isinstance(alloc, mybir.MemoryLocationSet):
            continue
        name = alloc.memorylocations[0].name
        if alloc.kind == "ExternalInput":
            if name != partition_name:
                in_names.append(name)
        elif alloc.kind == "ExternalOutput":
            out_names.append(name)
            shape = tuple(alloc.tensor_shape)
            dtype = mybir.dt.np(alloc.dtype)
            out_avals.append(jax.core.ShapedArray(shape, dtype))
            zero_outs.append(_np.zeros(shape, dtype))
    n_params = len(in_names)
    all_in_names = in_names + out_names
    if partition_name is not None:
        all_in_names.append(partition_name)

    def _body(*args):
        operands = list(args)
        if partition_name is not None:
            operands.append(partition_id_tensor())
        return tuple(_bass_exec_p.bind(
            *operands,
            out_avals=tuple(out_avals),
            in_names=tuple(all_in_names),
            out_names=tuple(out_names),
            lowering_input_output_aliases=(),
            sim_require_finite=True,
            sim_require_nnan=True,
            nc=nc,
        ))

    devices = jax.devices()[:N_CORES]
    mesh = Mesh(_np.asarray(devices), ("core",))
    nin = n_params + len(out_names)
    sharded = jax.jit(shard_map(
        _body, mesh=mesh,
        in_specs=(PartitionSpec("core"),) * nin,
        out_specs=(PartitionSpec("core"),) * len(out_names),
        check_rep=False))
    _RUNNER_CACHE[repeat] = {
        "fn": sharded, "in_names": in_names, "out_names": out_names,
        "zero_outs": zero_outs, "mesh": mesh,
    }
    return _RUNNER_CACHE[repeat]


def run_cores(in_maps):
    """Run the 8-core kernel on a list of per-core input dicts."""
    import numpy as _np
    r = get_runner()
    concat_in = [
        _np.concatenate([in_maps[c][name] for c in range(N_CORES)], axis=0)
        for name in r["in_names"]
    ] + [_np.concatenate([z] * N_CORES, axis=0) for z in r["zero_outs"]]
    outs = r["fn"](*concat_in)
    res = []
    for c in range(N_CORES):
        d = {}
        for i, name in enumerate(r["out_names"]):
            full = _np.asarray(outs[i])
            rows = full.shape[0] // N_CORES
            d[name] = full[c * rows:(c + 1) * rows]
        res.append(d)
    return res


def make_in_maps(dist, atomtypes, W1, b1, W2, b2, gw, gb):
    dist = np.asarray(dist, np.float32)
    atomtypes = np.asarray(atomtypes, np.float32)
    B, N, _, _ = atomtypes.shape
    P = B * N
    assert P == N_CORES * PC

    lhsT1, lhsT2, G1, cst = _pack_weights(
        np.asarray(W1, np.float32), np.asarray(b1, np.float32),
        np.asarray(W2, np.float32), np.asarray(b2, np.float32),
        np.asarray(gw, np.float32), np.asarray(gb, np.float32))

    # X [128, PP] per core, rows matching F exactly: per half h:
    # 64h+0..5 emb=1.0, 64h+6 ones=1.0, 64h+7 pad, 64h+8+6k+c atom
    # (neighbors 8h..8h+7), 64h+56+k dist.
    atom = atomtypes.reshape(P, K, D)
    dst = dist.reshape(P, K)
    x = np.zeros((P, 128), np.float32)
    for h in range(2):
        x[:, 64 * h:64 * h + 7] = 1.0
        x[:, 64 * h + 8:64 * h + 56] = atom[:, 8 * h:8 * h + 8, :].reshape(P, 48)
        x[:, 64 * h + 56:64 * h + 64] = dst[:, 8 * h:8 * h + 8]

    in_maps = []
    for c in range(N_CORES):
        sl = slice(c * PC, (c + 1) * PC)
        xa = np.zeros((PP, 128), np.float32)
        xa[:PC] = x[sl]
        in_maps.append({
            "x": np.ascontiguousarray(xa.T),
            "lhsT1": lhsT1, "lhsT2": lhsT2, "g1": G1, "cst": cst,
        })
    return in_maps, (B, N)


def _unscramble(o):
    """[84, NB*T] per core -> [PP, 6] points: row 6j+c col b*T+t is point
    b*GB*T + j*T + t channel c."""
    o = o.reshape(GB, 6, NB, T)            # [j, c, b, t]
    return o.transpose(2, 0, 3, 1).reshape(PP, 6)


def kernel(dist, atomtypes, W1, b1, W2, b2, gw, gb):
    in_maps, (B, N) = make_in_maps(dist, atomtypes, W1, b1, W2, b2, gw, gb)
    res = run_cores(in_maps)
    outs = [_unscramble(res[c]["out"])[:PC] for c in range(N_CORES)]
    return np.concatenate(outs, axis=0).reshape(B, N, D).astype(np.float32)


if __name__ == "__main__":
    rng = np.random.default_rng(0)
    inputs = {
        "dist": rng.random((4, 100000, 16, 1), dtype=np.float32),
        "atomtypes": rng.random((4, 100000, 16, 6), dtype=np.float32),
        "W1": rng.random((3, 13, 13), dtype=np.float32) - 0.5,
        "b1": rng.random((3, 13), dtype=np.float32) - 0.5,
        "W2": rng.random((3, 13, 6), dtype=np.float32) - 0.5,
        "b2": rng.random((3, 6), dtype=np.float32) - 0.5,
        "gw": np.ones((3, 6), np.float32),
        "gb": np.zeros((3, 6), np.float32),
    }
    out = kernel(**inputs)
    print(out.shape, out.dtype)


# revision 29
# speedup vs baseline: 2.1531x; 1.0743x over previous
"""Trainium2 Bass kernel: 3-layer GNN message passing (atom embedding).

Data-parallel over the B*N=400000 point axis across 8 NeuronCores.

Layout (channels-on-partitions, all-fp32):
  F [128, GB*T] per batch of GB=14 point-tiles; each 64-partition half h
  holds neighbors 8h..8h+7 of every point:
    rows 64h+0..5   emb (refreshed per layer from E)
    row  64h+6      const 1.0 (folds b1 into matmul1)
    rows 64h+8+6k+c atom   (k=0..7 within half)
    rows 64h+56+k   dist
  matmul1 per tile: two K=64 matmuls (identical lhsT1 content replicated
  at partitions 0-63 / 64-127) -> Z [104, 2T] PSUM.  LeakyReLU
  evacuation Z->H split between ScalarE (Prelu, cols 0:C1) and VectorE
  (copy to SBUF then (x*0.2) max x; DVE reads PSUM once per op).
  matmul2: per-tile lhsT2 [104,84] (W2 at output cols 6j) accumulates
  all GB tiles' messages directly into one batched PSUM stage [84, T];
  the +16*b2 bias is folded into the two PSUM->SBUF reads.  GroupNorm
  runs batched over [84, T]; mu/m2 via a block-averaging matmul whose
  PSUM tile borrows a Z-pool slot so two batches' chains never
  serialize on one buffer.

  Precision: GroupNorm var = m2 - mu^2 reaches ~6e-7 (below eps=1e-5)
  at layer 2 on this data, amplifying msg perturbations ~300x.  ANY
  16-bit quantization anywhere fails the 2e-2 tolerance (bf16 -> rel
  0.3, fp16 -> 0.23, fp32r/TF32-ish -> fails too; all measured), so
  every matmul stays plain fp32 (4 cycles/row).  PE is then ~94% busy
  and is the device-time floor (~1.05 ms/core).

  Batches are software-pipelined in groups of 2-3 so one batch's serial
  GN chain overlaps another's tile crunch.  ~8 DMAs per batch: one
  [128, GB*T] input load, out-store of E ([84, NB*T] layout,
  unscrambled on the host), and a per-layer emb refresh that bounces
  E through a DRAM scratch tile — partition-crossing (j c)->(c j)
  access patterns only lower correctly on the flat DRAM side of a DMA
  (the SBUF-side rearrange reads out of bounds; found via CoreSim),
  with explicit add_dep_helper edges because Tile's subtile tracking
  misses these APs.

  The per-dispatch bass2jax/axon overhead is ~2.0 ms per call (an empty
  kernel measures 2086 us), so _build_nc(repeat=R) can wrap the whole
  body in a hardware For_i loop: one dispatch then runs R full
  executions and test.py reports wall/R, i.e. the marginal kernel time.
"""
import sys

sys.path.insert(0, "/opt/trn_rl_repo")

import numpy as np

D = 6
K = 16
N_LAYERS = 3
C_IN = 13
EPS = 1e-5
SLOPE = 0.2

N_CORES = 8
T = 512            # points per tile (PSUM bank = 512 fp32)
PC = 50000         # points per core
NT = 98            # tiles per core
PP = NT * T        # padded points per core = 50176
GB = 14            # point-tiles per groupnorm batch (84 partitions)
SROWS = 6 * GB     # 84
C1 = 736           # ScalarE/VectorE split of the Z->H evacuation
NB = NT // GB      # 7 batches
# batch groups for software pipelining (2-3 batches in flight)
GROUPS = [(0, 1), (2, 3), (4, 5, 6)]
USE_F32R = False


def _pack_weights(W1, b1, W2, b2, gw, gb):
    """Build packed lhsT / const tensors (host side, a few KB)."""
    # lhsT1 [128, 3*104]: rows 0-63 = half template, 64-127 copy.
    half = np.zeros((64, N_LAYERS * 104), np.float32)
    for i in range(N_LAYERS):
        for k8 in range(8):
            cols = slice(i * 104 + k8 * 13, i * 104 + k8 * 13 + 13)
            half[0:6, cols] = W1[i, 0:6, :]          # emb rows
            half[6, cols] = b1[i]                    # ones row -> bias
            half[8 + k8 * 6:8 + k8 * 6 + 6, cols] = W1[i, 6:12, :]
            half[56 + k8, cols] = W1[i, 12, :]       # dist row
    lhsT1 = np.concatenate([half, half], axis=0)     # [128, 312]

    # lhsT2 [104, 3*GB*84]: variant (i, j) puts W2[i] at output cols
    # 6j..6j+6 for every neighbor block.
    lhsT2 = np.zeros((104, N_LAYERS * GB * 84), np.float32)
    for i in range(N_LAYERS):
        for j in range(GB):
            base = (i * GB + j) * 84
            for k8 in range(8):
                lhsT2[k8 * 13:k8 * 13 + 13,
                      base + 6 * j:base + 6 * j + 6] = W2[i]

    G1 = np.kron(np.eye(2 * GB, dtype=np.float32),
                 np.ones((3, 3), np.float32) / 3.0)  # [84, 84]

    cst = np.zeros((6, 9), np.float32)
    for i in range(N_LAYERS):
        cst[:, i] = 16.0 * b2[i]
        cst[:, 3 + i] = gw[i]
        cst[:, 6 + i] = gb[i]
    cst = np.tile(cst, (GB, 1))  # [84, 9]
    return lhsT1, lhsT2, G1, cst


def _build_nc(pp=PP, repeat=1):
    import concourse.bass as bass
    import concourse.bacc as bacc
    import concourse.mybir as mybir
    from concourse import tile
    from concourse.tile_rust import add_dep_helper as _adh

    def add_dep_helper(a, b, reason=""):
        _adh(a.ins, b.ins, reason=reason)

    bass_ds = bass.ds
    F32 = mybir.dt.float32
    F32R = mybir.dt.float32r
    AF = mybir.ActivationFunctionType
    OP = mybir.AluOpType

    MD = F32R if USE_F32R else F32

    import time as _time
    print(f"[kernel] build start {_time.time():.1f}", flush=True)
    nc = bacc.Bacc("TRN2", target_bir_lowering=False)
    x_e = nc.declare_dram_parameter("x", [128, pp], MD, isOutput=False)
    l1_e = nc.declare_dram_parameter("lhsT1", [128, N_LAYERS * 104], MD,
                                     isOutput=False)
    l2_e = nc.declare_dram_parameter("lhsT2", [104, N_LAYERS * GB * 84], MD,
                                     isOutput=False)
    g1_e = nc.declare_dram_parameter("g1", [SROWS, SROWS], F32, isOutput=False)
    cst_e = nc.declare_dram_parameter("cst", [SROWS, 9], F32, isOutput=False)
    out_e = nc.declare_dram_parameter("out", [SROWS, (pp // (GB * T)) * T], F32,
                                      isOutput=True)

    GBT = GB * T
    nb = pp // GBT
    if nb == NB:
        groups = GROUPS
    else:
        groups = [tuple(range(nb))]

    with tile.TileContext(nc) as tc:
        with tc.tile_pool(name="w", bufs=1) as wp, \
             tc.tile_pool(name="f", bufs=3) as fp, \
             tc.tile_pool(name="e", bufs=3) as ep, \
             tc.tile_pool(name="h", bufs=8) as hp, \
             tc.tile_pool(name="g", bufs=3) as gp, \
             tc.tile_pool(name="z", bufs=3, space="PSUM") as zp, \
             tc.tile_pool(name="s", bufs=2, space="PSUM") as sp, \
             tc.tile_pool(name="d", bufs=4, space="DRAM") as dp:
            l1 = wp.tile([128, N_LAYERS * 104], MD)
            l2 = wp.tile([104, N_LAYERS * GB * 84], MD)
            g1 = wp.tile([SROWS, SROWS], F32)
            cst = wp.tile([SROWS, 9], F32)
            eps = wp.tile([128, 1], F32)
            nc.sync.dma_start(out=l1[:], in_=l1_e[:])
            nc.sync.dma_start(out=l2[:], in_=l2_e[:])
            nc.sync.dma_start(out=g1[:], in_=g1_e[:])
            nc.sync.dma_start(out=cst[:], in_=cst_e[:])
            nc.gpsimd.memset(eps[:], EPS)

            Fs, Es = {}, {}
            eadd, mm1_last = {}, {}

            def preamble(b):
                F = fp.tile([128, GBT], MD, tag="F")
                nc.sync.dma_start(out=F[:], in_=x_e[:, bass_ds(b * GBT, GBT)])
                E = ep.tile([SROWS, T], F32, tag="E")
                nc.vector.memset(E[:], 1.0)
                Fs[b], Es[b] = F, E

            def unit(b, i):
                F, E = Fs[b], Es[b]
                refr = []
                if i > 0:
                    # Partition-crossing APs only lower correctly on the
                    # DRAM side, so bounce E through a DRAM scratch: the
                    # (j c)->(c j) partition mix happens in flat DRAM.
                    Ed = dp.tile([SROWS, T], F32, tag="Ed")
                    hop1 = nc.gpsimd.dma_start(out=Ed[:], in_=E[:])
                    add_dep_helper(hop1, eadd[b], reason="refresh RAW E")
                    esrc = Ed[:].rearrange("(j c) t -> c j t", j=GB, c=6)
                    for h in range(2):
                        dst = F[64 * h:64 * h + 6, :].rearrange(
                            "c (j t) -> c j t", j=GB, t=T)
                        d = nc.gpsimd.dma_start(out=dst, in_=esrc)
                        add_dep_helper(d, hop1, reason="refresh RAW Edram")
                        add_dep_helper(d, mm1_last[b], reason="refresh WAR F")
                        refr.append(d)
                stage = sp.tile([SROWS, T], F32, tag="stage")
                for j in range(GB):
                    Z = zp.tile([128, 2 * T], F32, tag="Z")
                    m1a = nc.tensor.matmul(Z[0:104, 0:T],
                                     l1[0:64, i * 104:(i + 1) * 104],
                                     F[0:64, bass_ds(j * T, T)],
                                     start=True, stop=True)
                    if j == 0:
                        for d in refr:
                            add_dep_helper(m1a, d, reason="MM1 RAW refresh")
                    m1b = nc.tensor.matmul(Z[0:104, T:2 * T],
                                     l1[64:128, i * 104:(i + 1) * 104],
                                     F[64:128, bass_ds(j * T, T)],
                                     start=True, stop=True)
                    mm1_last[b] = m1b
                    H = hp.tile([104, 2 * T], MD, tag="H")
                    nc.scalar.activation(H[:, 0:C1], Z[0:104, 0:C1],
                                         AF.Prelu, bias=0.0, scale=1.0,
                                         alpha=SLOPE)
                    # DVE reads PSUM only once per op: copy to SBUF,
                    # then lrelu = (x*0.2) max x.
                    tmp = hp.tile([104, 2 * T - C1], F32, tag="tmp")
                    nc.vector.tensor_copy(tmp[:], Z[0:104, C1:2 * T])
                    nc.vector.scalar_tensor_tensor(
                        H[:, C1:2 * T], tmp[:], SLOPE, tmp[:],
                        OP.mult, OP.max)
                    w2 = l2[:, (i * GB + j) * 84:(i * GB + j + 1) * 84]
                    nc.tensor.matmul(stage[:], w2, H[:, 0:T],
                                     start=(j == 0), stop=False)
                    nc.tensor.matmul(stage[:], w2, H[:, T:2 * T],
                                     start=False, stop=(j == GB - 1))

                # ---- batched GroupNorm over [84, T], fp32 ----
                bias = cst[0:SROWS, i:i + 1]
                sq = gp.tile([SROWS, T], F32, tag="sq")
                nc.scalar.activation(sq[:], stage[:], AF.Square, bias=bias,
                                     scale=1.0)
                st = gp.tile([SROWS, T], F32, tag="st")
                nc.vector.tensor_scalar(st[:], stage[:], bias, None, OP.add)
                # mu/m2 borrow a Z-pool slot (same shape) so two batches'
                # GN chains never serialize on a dedicated single buffer.
                mum2 = zp.tile([128, 2 * T], F32, tag="Z")
                nc.tensor.matmul(mum2[0:SROWS, 0:T], g1[:], st[:],
                                 start=True, stop=True)
                nc.tensor.matmul(mum2[0:SROWS, T:2 * T], g1[:], sq[:],
                                 start=True, stop=True)
                mu = mum2[0:SROWS, 0:T]
                m2 = mum2[0:SROWS, T:2 * T]
                mus2 = gp.tile([SROWS, T], F32, tag="mus2")
                nc.scalar.activation(mus2[:], mu, AF.Square)
                vpe = gp.tile([SROWS, T], F32, tag="vpe")
                nc.vector.scalar_tensor_tensor(
                    vpe[:], mus2[:], -1.0, m2, OP.mult, OP.add)
                rstd = gp.tile([SROWS, T], F32, tag="rstd")
                nc.scalar.activation(rstd[:], vpe[:],
                                     AF.Abs_reciprocal_sqrt,
                                     bias=eps[0:SROWS, 0:1], scale=1.0)
                y = gp.tile([SROWS, T], F32, tag="y")
                nc.vector.scalar_tensor_tensor(
                    y[:], mu, -1.0, st[:], OP.mult, OP.add)
                yr = gp.tile([SROWS, T], F32, tag="yr")
                nc.vector.tensor_mul(yr[:], y[:], rstd[:])
                upd = gp.tile([SROWS, T], F32, tag="upd")
                nc.scalar.activation(upd[:], yr[:], AF.Prelu,
                                     bias=cst[:, 6 + i:7 + i],
                                     scale=cst[:, 3 + i:4 + i], alpha=SLOPE)
                eadd[b] = nc.vector.tensor_add(E[:], E[:], upd[:])

            def outstore(b):
                E = Es[b]
                # gpsimd (SWDGE) keeps the out-store off the SP HWDGE
                # FIFO so it never head-of-line blocks the next loads.
                d = nc.gpsimd.dma_start(
                    out=out_e[:, bass_ds(b * T, T)], in_=E[:])
                add_dep_helper(d, eadd[b], reason="out RAW E")

            def whole():
                for grp in groups:
                    for b in grp:
                        preamble(b)
                    for i in range(N_LAYERS):
                        for b in grp:
                            unit(b, i)
                    for b in grp:
                        outstore(b)

            if repeat > 1:
                # In-NEFF repeat loop: amortizes the ~2 ms per-dispatch
                # axon/PJRT overhead across `repeat` full executions so
                # the marginal kernel time is measurable.
                with tc.For_i(0, repeat, 1):
                    whole()
            else:
                whole()

    print(f"[kernel] trace done {_time.time():.1f}", flush=True)
    nc.compile()
    print(f"[kernel] bacc compile done {_time.time():.1f}", flush=True)
    return nc


_RUNNER_CACHE = {}


def get_runner(repeat=1):
    """Build (once per repeat) the jitted 8-core executable. Returns a dict:
    fn(concat_inputs...) -> out jax arrays, in_names order, zero_outs."""
    global _RUNNER_CACHE
    if repeat in _RUNNER_CACHE:
        return _RUNNER_CACHE[repeat]
    import jax
    import numpy as _np
    from jax.sharding import Mesh, PartitionSpec
    from jax.experimental.shard_map import shard_map
    import concourse.mybir as mybir
    from concourse.bass2jax import (
        install_neuronx_cc_hook, _bass_exec_p, partition_id_tensor)

    nc = _build_nc(repeat=repeat)
    install_neuronx_cc_hook()
    partition_name = nc.partition_id_tensor.name if nc.partition_id_tensor else None
    in_names, out_names, out_avals, zero_outs = [], [], [], []
    for alloc in nc.m.functions[0].allocations:
        if not isinstance(alloc, mybir.MemoryLocationSet):
            continue
        name = alloc.memorylocations[0].name
        if alloc.kind == "ExternalInput":
            if name != partition_name:
                in_names.append(name)
        elif alloc.kind == "ExternalOutput":
            out_names.append(name)
            shape = tuple(alloc.tensor_shape)
            dtype = mybir.dt.np(alloc.dtype)
            out_avals.append(jax.core.ShapedArray(shape, dtype))
            zero_outs.append(_np.zeros(shape, dtype))
    n_params = len(in_names)
    all_in_names = in_names + out_names
    if partition_name is not None:
        all_in_names.append(partition_name)

    def _body(*args):
        operands = list(args)
        if partition_name is not None:
            operands.append(partition_id_tensor())
        return tuple(_bass_exec_p.bind(
            *operands,
            out_avals=tuple(out_avals),
            in_names=tuple(all_in_names),
            out_names=tuple(out_names),
            lowering_input_output_aliases=(),
            sim_require_finite=True,
            sim_require_nnan=True,
            nc=nc,
        ))

    devices = jax.devices()[:N_CORES]
    mesh = Mesh(_np.asarray(devices), ("core",))
    nin = n_params + len(out_names)
    sharded = jax.jit(shard_map(
        _body, mesh=mesh,
        in_specs=(PartitionSpec("core"),) * nin,
        out_specs=(PartitionSpec("core"),) * len(out_names),
        check_rep=False))
    _RUNNER_CACHE[repeat] = {
        "fn": sharded, "in_names": in_names, "out_names": out_names,
        "zero_outs": zero_outs, "mesh": mesh,
    }
    return _RUNNER_CACHE[repeat]


def run_cores(in_maps):
    """Run the 8-core kernel on a list of per-core input dicts."""
    import numpy as _np
    r = get_runner()
    concat_in = [
        _np.concatenate([in_maps[c][name] for c in range(N_CORES)], axis=0)
        for name in r["in_names"]
    ] + [_np.concatenate([z] * N_CORES, axis=0) for z in r["zero_outs"]]
    outs = r["fn"](*concat_in)
    res = []
    for c in range(N_CORES):
        d = {}
        for i, name in enumerate(r["out_names"]):
            full = _np.asarray(outs[i])
            rows = full.shape[0] // N_CORES
            d[name] = full[c * rows:(c + 1) * rows]
        res.append(d)
    return res


def make_in_maps(dist, atomtypes, W1, b1, W2, b2, gw, gb):
    dist = np.asarray(dist, np.float32)
    atomtypes = np.asarray(atomtypes, np.float32)
    B, N, _, _ = atomtypes.shape
    P = B * N
    assert P == N_CORES * PC

    lhsT1, lhsT2, G1, cst = _pack_weights(
        np.asarray(W1, np.float32), np.asarray(b1, np.float32),
        np.asarray(W2, np.float32), np.asarray(b2, np.float32),
        np.asarray(gw, np.float32), np.asarray(gb, np.float32))

    # X [128, PP] per core, rows matching F exactly: per half h:
    # 64h+0..5 emb=1.0, 64h+6 ones=1.0, 64h+7 pad, 64h+8+6k+c atom
    # (neighbors 8h..8h+7), 64h+56+k dist.
    atom = atomtypes.reshape(P, K, D)
    dst = dist.reshape(P, K)
    x = np.zeros((P, 128), np.float32)
    for h in range(2):
        x[:, 64 * h:64 * h + 7] = 1.0
        x[:, 64 * h + 8:64 * h + 56] = atom[:, 8 * h:8 * h + 8, :].reshape(P, 48)
        x[:, 64 * h + 56:64 * h + 64] = dst[:, 8 * h:8 * h + 8]

    in_maps = []
    for c in range(N_CORES):
        sl = slice(c * PC, (c + 1) * PC)
        xa = np.zeros((PP, 128), np.float32)
        xa[:PC] = x[sl]
        in_maps.append({
            "x": np.ascontiguousarray(xa.T),
            "lhsT1": lhsT1, "lhsT2": lhsT2, "g1": G1, "cst": cst,
        })
    return in_maps, (B, N)


def _unscramble(o):
    """[84, NB*T] per core -> [PP, 6] points: row 6j+c col b*T+t is point
    b*GB*T + j*T + t channel c."""
    o = o.reshape(GB, 6, NB, T)            # [j, c, b, t]
    return o.transpose(2, 0, 3, 1).reshape(PP, 6)


def kernel(dist, atomtypes, W1, b1, W2, b2, gw, gb):
    in_maps, (B, N) = make_in_maps(dist, atomtypes, W1, b1, W2, b2, gw, gb)
    res = run_cores(in_maps)
    outs = [_unscramble(res[c]["out"])[:PC] for c in range(N_CORES)]
    return np.concatenate(outs, axis=0).reshape(B, N, D).astype(np.float32)


if __name__ == "__main__":
    rng = np.random.default_rng(0)
    inputs = {
        "dist": rng.random((4, 100000, 16, 1), dtype=np.float32),
        "atomtypes": rng.random((4, 100000, 16, 6), dtype=np.float32),
        "W1": rng.random((3, 13, 13), dtype=np.float32) - 0.5,
        "b1": rng.random((3, 13), dtype=np.float32) - 0.5,
        "W2": rng.random((3, 13, 6), dtype=np.float32) - 0.5,
        "b2": rng.random((3, 6), dtype=np.float32) - 0.5,
        "gw": np.ones((3, 6), np.float32),
        "gb": np.zeros((3, 6), np.float32),
    }
    out = kernel(**inputs)
    print(out.shape, out.dtype)
